# revision 1
# baseline (speedup 1.0000x reference)
"""MeanStdFilter kernel for 8 Trainium2 NeuronCores.

Semantics (matches the sequential-Welford reference with M=0, S=S_in, n=0):
    S1[f] = sum_b x[b, f]            (global, over all 32768 rows)
    S2[f] = sum_b x[b, f]^2
    mean  = S1 / N
    M2    = S2 - S1^2 / N + S_in     (Welford M2 started from buffer S)
    var   = M2 / (N - 1)             (N = 32768 > 1)
    out   = (x - mean) / (sqrt(var) + 1e-5)
The input running-mean buffer M is overwritten by the first Welford step in
the reference, so it never affects the output.

Distribution: x is sharded 4096 rows per core. Each core keeps its shard
resident in SBUF (4 contiguous chunks of 8 row-tiles), computes partial raw
sums, AllReduces 8 KB of stats, finalizes redundantly on every core in a
packed [128,8] layout, then normalizes IN PLACE and stores. HBM traffic per
core = one 16.8 MB read + one 16.8 MB write.

Engine balance (HW-measured):
  - fp32 matmul streams ~2.4 ns/col -> ones-matmul 2.46+ us per [128,1024]
    tile; DVE fp32 tensor_tensor 1.22 us per tile. S1 split: 19 tiles on
    PE, 13 on DVE (acc1 chain), merged into the PSUM group at the end.
  - Concurrent GpSimd tensor_tensor degrades DVE 1.22 -> 3.3 us (shared
    SBUF port mux): normalize runs on DVE only, as 8 chunked ops of
    FD=8192 (amortizes the 151-cycle DVE instruction overhead).
  - Warmup AllReduce at kernel start primes CC rings / absorbs start skew.
"""

import functools

import numpy as np

import concourse.bacc as bacc
import concourse.tile as tile
from concourse import mybir
from concourse.bass_utils import run_bass_kernel_spmd

NCORES = 8
B, F = 32768, 1024
ROWS = B // NCORES  # 4096 rows per core
P = 128
NT = ROWS // P  # 32 row-tiles of [128, 1024] per core
TPC = 8  # tiles per resident chunk
NCHUNK = NT // TPC
EPS = 1e-5
FP32 = mybir.dt.float32
AF = mybir.ActivationFunctionType
ALU = mybir.AluOpType

# Tiles whose S1 contribution is accumulated on DVE instead of PE (13 of 32).
DVE_S1_TILES = frozenset(t for t in range(NT) if t % 5 in (1, 3))


def build_kernel():
    nc = bacc.Bacc(
        "TRN2", target_bir_lowering=False, debug=False, num_devices=NCORES
    )
    x = nc.declare_dram_parameter("x", [ROWS, F], FP32, isOutput=False)
    s_in = nc.declare_dram_parameter("S", [1, F], FP32, isOutput=False)
    out = nc.declare_dram_parameter("out", [ROWS, F], FP32, isOutput=True)

    x_t = x[:].rearrange("(n p) f -> n p f", p=P)
    out_t = out[:].rearrange("(n p) f -> n p f", p=P)
    groups = [list(range(NCORES))]

    with tile.TileContext(nc) as tc:
        with (
            tc.tile_pool(name="xbuf", bufs=1) as xpool,
            tc.tile_pool(name="work", bufs=3) as work,
            tc.tile_pool(name="stats", bufs=1) as stats,
            tc.tile_pool(name="psum", bufs=1, space="PSUM") as psum,
            tc.tile_pool(name="dram", bufs=1, space="DRAM") as dram,
        ):
            # Warmup AllReduce: primes the CC rings and synchronizes core
            # start skew while the load phase runs. Result is unused.
            wu = stats.tile([1, 8], FP32)
            nc.vector.memset(wu, 0.0)
            wu_in = dram.tile([1, 8], FP32)
            wu_out = dram.tile([1, 8], FP32)
            nc.sync.dma_start(out=wu_in[:], in_=wu[:])
            nc.gpsimd.collective_compute(
                "AllReduce",
                ALU.add,
                replica_groups=groups,
                ins=[wu_in[:].opt()],
                outs=[wu_out[:].opt()],
            )

            ones = stats.tile([P, 1], FP32)
            nc.vector.memset(ones, 1.0)
            accsq = stats.tile([P, F], FP32)
            acc1 = stats.tile([P, F], FP32)

            # Resident shard: 4 chunks x [128, 8, 1024] (32 KB/partition each).
            xb = [
                xpool.tile([P, TPC, F], FP32, tag=f"xb{c}", name=f"xb{c}")
                for c in range(NCHUNK)
            ]

            def xtile(t):
                return xb[t // TPC][:, t % TPC, :]

            # One PSUM bank per 512-wide half (fp32 matmul N<=512/bank).
            ps1 = [psum.tile([1, 512], FP32, tag=f"ps1_{h}", name=f"ps1_{h}") for h in range(2)]
            ps2 = [psum.tile([1, 512], FP32, tag=f"ps2_{h}", name=f"ps2_{h}") for h in range(2)]

            # ---- Phase A: load shard, accumulate raw sums ----
            first_dve = min(DVE_S1_TILES)
            first_pe = min(t for t in range(NT) if t not in DVE_S1_TILES)
            for t in range(NT):
                xt = xtile(t)
                nc.sync.dma_start(out=xt, in_=x_t[t])
                if t in DVE_S1_TILES:
                    if t == first_dve:
                        nc.vector.tensor_copy(acc1[:], xt)
                    else:
                        nc.vector.tensor_tensor(acc1[:], acc1, xt, ALU.add)
                else:
                    for h in range(2):
                        nc.tensor.matmul(
                            ps1[h][:],
                            lhsT=ones[:],
                            rhs=xt[:, h * 512 : (h + 1) * 512],
                            start=(t == first_pe),
                            stop=False,
                        )
                sq = work.tile([P, F], FP32, tag="sq")
                nc.scalar.activation(sq, xt, AF.Square)
                if t == 0:
                    nc.vector.tensor_copy(accsq[:], sq)
                else:
                    nc.vector.tensor_tensor(accsq[:], accsq, sq, ALU.add)

            # Merge the DVE-side S1 partial into the PSUM accumulation group,
            # and reduce accsq across partitions.
            for h in range(2):
                nc.tensor.matmul(
                    ps1[h][:],
                    lhsT=ones[:],
                    rhs=acc1[:, h * 512 : (h + 1) * 512],
                    start=False,
                    stop=True,
                )
                nc.tensor.matmul(
                    ps2[h][:],
                    lhsT=ones[:],
                    rhs=accsq[:, h * 512 : (h + 1) * 512],
                    start=True,
                    stop=True,
                )

            cc_stage = stats.tile([1, 2 * F], FP32)
            for h in range(2):
                nc.scalar.copy(cc_stage[:, h * 512 : (h + 1) * 512], ps1[h][:])
                nc.scalar.copy(
                    cc_stage[:, F + h * 512 : F + (h + 1) * 512], ps2[h][:]
                )

            cc_in = dram.tile([1, 2 * F], FP32)
            cc_out = dram.tile([1, 2 * F], FP32)
            nc.sync.dma_start(out=cc_in[:], in_=cc_stage[:])
            nc.gpsimd.collective_compute(
                "AllReduce",
                ALU.add,
                replica_groups=groups,
                ins=[cc_in[:].opt()],
                outs=[cc_out[:].opt()],
            )

            # ---- Packed finalize: [128, 8] per-feature layout (f = p*8+j).
            # All FD-8 ops, so the whole chain is ~2us instead of ~20us.
            s12p = stats.tile([P, 2, 8], FP32)
            nc.sync.dma_start(
                out=s12p[:],
                in_=cc_out[:].rearrange("a (h p j) -> a p h j", h=2, p=P, j=8),
            )
            sinp = stats.tile([P, 8], FP32)
            nc.sync.dma_start(
                out=sinp[:], in_=s_in[:].rearrange("a (p j) -> a p j", p=P, j=8)
            )

            s1v = s12p[:, 0, :]
            s2v = s12p[:, 1, :]
            mr = stats.tile([P, 16], FP32)  # cols 0:8 mean, 8:16 rstd
            finw = stats.tile([P, 32], FP32)
            w1, w2, w3, w4 = (finw[:, 8 * i : 8 * (i + 1)] for i in range(4))
            nc.scalar.activation(mr[:, 0:8], s1v, AF.Copy, scale=1.0 / B)
            nc.vector.tensor_tensor(w1, s1v, mr[:, 0:8], ALU.mult)  # S1^2/N
            nc.vector.tensor_tensor(w2, s2v, w1, ALU.subtract)  # M2
            nc.vector.tensor_tensor(w2, w2, sinp[:], ALU.add)  # + S_in
            nc.scalar.activation(w3, w2, AF.Sqrt, scale=1.0 / (B - 1))  # std
            nc.scalar.activation(w4, w3, AF.Copy, bias=EPS)  # std + eps
            nc.vector.reciprocal(mr[:, 8:16], w4)

            # Round-trip through DRAM to broadcast per-feature mean/rstd
            # across all 128 partitions ([128,16] row-major == feature order).
            mr_d = dram.tile([1, 2 * F], FP32)
            nc.sync.dma_start(
                out=mr_d[:].rearrange("a (h p j) -> a p h j", h=2, p=P, j=8),
                in_=mr[:].rearrange("p (h j) -> p h j", h=2, j=8),
            )
            mean_b = stats.tile([P, F], FP32)
            rstd_b = stats.tile([P, F], FP32)
            nc.sync.dma_start(out=mean_b[:], in_=mr_d[:, 0:F].to_broadcast([P, F]))
            nc.sync.dma_start(
                out=rstd_b[:], in_=mr_d[:, F : 2 * F].to_broadcast([P, F])
            )

            # ---- Phase C: normalize in place, chunked (FD=8192 per op) ----
            for c in range(NCHUNK):
                mb = mean_b[:, None, :].to_broadcast([P, TPC, F])
                rb = rstd_b[:, None, :].to_broadcast([P, TPC, F])
                nc.vector.tensor_tensor(xb[c][:], xb[c], mb, ALU.subtract)
                nc.vector.tensor_tensor(xb[c][:], xb[c], rb, ALU.mult)
                for j in range(TPC):
                    t = c * TPC + j
                    nc.sync.dma_start(out=out_t[t], in_=xb[c][:, j, :])

    nc.finalize()
    return nc


@functools.cache
def _get_nc():
    return build_kernel()


def kernel(x, M, S, _trace=False, _trace_kwargs=None):
    del M  # overwritten by the first Welford step in the reference
    x = np.ascontiguousarray(x, dtype=np.float32)
    S = np.ascontiguousarray(S, dtype=np.float32).reshape(1, F)
    nc = _get_nc()
    in_maps = [
        {"x": x[i * ROWS : (i + 1) * ROWS], "S": S} for i in range(NCORES)
    ]
    res = run_bass_kernel_spmd(
        nc,
        in_maps,
        core_ids=list(range(NCORES)),
        trace=_trace,
        **(_trace_kwargs or {}),
    )
    out = np.concatenate([res.results[i]["out"] for i in range(NCORES)], axis=0)
    if _trace:
        return out, res
    return out



# revision 3
# speedup vs baseline: 1.0748x; 1.0748x over previous
"""MeanStdFilter kernel for 8 Trainium2 NeuronCores (v3).

Semantics (matches the sequential-Welford reference with M=0, S=S_in, n=0):
    S1[f] = sum_b x[b, f]            (global, over all 32768 rows)
    S2[f] = sum_b x[b, f]^2
    mean  = S1 / N
    var   = (S2 - S1^2/N + S_in) / (N - 1)
    out   = (x - mean) / (sqrt(var) + 1e-5)
The input running-mean buffer M is overwritten by the first Welford step in
the reference, so it never affects the output.

v3 architecture (vs the 200.5us v2 baseline):
  - Stats entirely on PE: per-chunk S1 via float32r ones-matmuls (1 cyc/row
    at free>=256 -> ~4x faster than fp32), S2 via fp16 ones-matmuls over
    Act-engine squares. DVE does no stats work.
  - Pre-AllReduce normalize pass1: y16 = x - m~_c (per-chunk mean broadcast)
    runs on DVE *during* the load phase, hidden under HBM traffic. The
    fp32 -> fp16 rounding happens after the subtract, so output error stays
    multiplicative in |out| (no blow-up of relative error near zero).
  - 4KB fp16 AllReduce payload, prescaled so cols 0:1024 are the global
    mean directly; the chunk totals are summed on DVE ([1,1024] adds).
  - Post-AR critical path: broadcast global mean -> delta_c = gmean - m~_c
    (all-fp16 DVE), then per chunk: y -= delta_c; y *= rstd (all-fp16 DVE
    2x mode), fp16 stores (half the write traffic; host upcasts).
  - rstd chain runs packed [128,8] off the critical path, concurrent with
    the first pass2 chunk. Sqrt activation table preloaded during phase A.
  - Warmup AllReduce kept: it pulls the one-time CC-init/skew barrier
    (~35us) into the load phase.
"""

import functools

import numpy as np

import concourse.bacc as bacc
import concourse.tile as tile
from concourse import mybir
from concourse.bass_utils import run_bass_kernel_spmd

NCORES = 8
B, F = 32768, 1024
ROWS = B // NCORES  # 4096 rows per core
P = 128
NT = ROWS // P  # 32 row-tiles of [128, 1024] per core
TPC = 8  # tiles per chunk
NCHUNK = NT // TPC  # 4
CROWS = P * TPC  # 1024 rows per chunk
EPS = 1e-5
FP32 = mybir.dt.float32
FP32R = mybir.dt.float32r
FP16 = mybir.dt.float16
AF = mybir.ActivationFunctionType
ALU = mybir.AluOpType


def build_kernel():
    nc = bacc.Bacc(
        "TRN2", target_bir_lowering=False, debug=False, num_devices=NCORES
    )
    x = nc.declare_dram_parameter("x", [ROWS, F], FP32, isOutput=False)
    s_in = nc.declare_dram_parameter("S", [1, F], FP32, isOutput=False)
    out = nc.declare_dram_parameter("out", [ROWS, F], FP16, isOutput=True)

    x_t = x[:].rearrange("(n p) f -> n p f", p=P)
    out_t = out[:].rearrange("(n p) f -> n p f", p=P)
    groups = [list(range(NCORES))]

    with tile.TileContext(nc) as tc:
        with (
            tc.tile_pool(name="xst", bufs=2) as xst_pool,
            tc.tile_pool(name="ybuf", bufs=1) as ybuf,
            tc.tile_pool(name="sq", bufs=3) as sqpool,
            tc.tile_pool(name="stats", bufs=1) as stats,
            tc.tile_pool(name="psum", bufs=1, space="PSUM") as psum,
            tc.tile_pool(name="dram", bufs=1, space="DRAM") as dram,
        ):
            # Warmup AllReduce: pulls the one-time CC-init / start-skew
            # barrier into the load phase. Result is unused.
            wu = stats.tile([1, 8], FP32)
            nc.vector.memset(wu, 0.0)
            wu_in = dram.tile([1, 8], FP32)
            wu_out = dram.tile([1, 8], FP32)
            nc.sync.dma_start(out=wu_in[:], in_=wu[:])
            nc.gpsimd.collective_compute(
                "AllReduce",
                ALU.add,
                replica_groups=groups,
                ins=[wu_in[:].opt()],
                outs=[wu_out[:].opt()],
            )

            ones = stats.tile([P, 1], FP32)
            nc.vector.memset(ones, 1.0)
            ones16 = stats.tile([P, 1], FP16)
            nc.vector.memset(ones16, 1.0)

            # Preload the Sqrt activation table so the post-AR rstd chain
            # doesn't pay the ~1.3us table swap on the critical path.
            dummy = stats.tile([1, 8], FP32)
            nc.scalar.activation(dummy, wu[:], AF.Sqrt)

            # S buffer, packed per-feature layout f = p*8 + j.
            sinp = stats.tile([P, 8], FP32)
            nc.sync.dma_start(
                out=sinp[:], in_=s_in[:].rearrange("a (p j) -> a p j", p=P, j=8)
            )

            # Resident normalized-intermediate shard (fp16), 64KB/partition.
            y16 = ybuf.tile([P, NT, F], FP16, name="y16")

            # Per-chunk S1 PSUM pair (reused across the 4 chunk groups) and
            # global S2 pair (one accumulation group over all 32 tiles).
            ps1 = [psum.tile([1, 512], FP32, tag=f"ps1_{h}", name=f"ps1_{h}") for h in range(2)]
            ps2 = [psum.tile([1, 512], FP32, tag=f"ps2_{h}", name=f"ps2_{h}") for h in range(2)]

            # Chunk mean staging: fp16 row, DRAM bounce, [128,F] broadcast.
            m16 = [stats.tile([1, F], FP16, name=f"m16_{c}") for c in range(NCHUNK)]
            s_c = [stats.tile([1, F], FP32, name=f"s_c{c}") for c in range(NCHUNK)]
            s_tot = stats.tile([1, F], FP32)
            md = dram.tile([NCHUNK, F], FP16)
            mb16 = [
                stats.tile([P, F], FP16, name=f"mb16_{c}") for c in range(NCHUNK)
            ]

            # ---- Phase A: stream chunks, stats on PE/Act, pass1 on DVE ----
            for c in range(NCHUNK):
                xc = xst_pool.tile([P, TPC, F], FP32, tag="xst", name=f"x_c{c}")
                for j in range(TPC):
                    t = c * TPC + j
                    xt = xc[:, j, :]
                    nc.sync.dma_start(out=xt, in_=x_t[t])
                    sq = sqpool.tile([P, F], FP16, tag="sq")
                    nc.scalar.activation(sq, xt, AF.Square)
                    for h in range(2):
                        hs = slice(h * 512, (h + 1) * 512)
                        nc.tensor.matmul(
                            ps1[h][:],
                            lhsT=ones[:],
                            rhs=xt[:, hs],
                            start=(j == 0),
                            stop=(j == TPC - 1),
                        )
                        nc.tensor.matmul(
                            ps2[h][:],
                            lhsT=ones16[:],
                            rhs=sq[:, hs],
                            start=(t == 0),
                            stop=(t == NT - 1),
                        )
                # Chunk stats: raw sums (for the AR payload) and the fp16
                # chunk mean m~_c (for pass1).
                for h in range(2):
                    hs = slice(h * 512, (h + 1) * 512)
                    nc.scalar.copy(s_c[c][:, hs], ps1[h][:])
                    nc.scalar.activation(
                        m16[c][:, hs], ps1[h][:], AF.Copy, scale=1.0 / CROWS
                    )
                # Accumulate the running total on DVE ahead of pass1 so the
                # AllReduce payload never waits on pass1 progress.
                if c == 0:
                    nc.vector.tensor_copy(s_tot[:], s_c[c])
                else:
                    nc.vector.tensor_tensor(s_tot[:], s_tot, s_c[c], ALU.add)
                # Broadcast m~_c to all partitions via a DRAM bounce.
                nc.sync.dma_start(out=md[c : c + 1, :], in_=m16[c][:])
                nc.sync.dma_start(
                    out=mb16[c][:], in_=md[c : c + 1, :].to_broadcast([P, F])
                )
                # pass1: y16 = x - m~_c  (fp32 math, fp16 result: the error
                # stays proportional to |x - m~_c|).
                mb = mb16[c][:, None, :].to_broadcast([P, TPC, F])
                nc.vector.tensor_tensor(
                    y16[:, c * TPC : (c + 1) * TPC, :], xc[:], mb, ALU.subtract
                )

            # ---- AllReduce: [mean | S2/(N-1)] in fp16, 4KB ----
            cc_stage = stats.tile([1, 2 * F], FP16)
            nc.scalar.activation(
                cc_stage[:, 0:F], s_tot[:], AF.Copy, scale=1.0 / B
            )
            for h in range(2):
                nc.scalar.activation(
                    cc_stage[:, F + h * 512 : F + (h + 1) * 512],
                    ps2[h][:],
                    AF.Copy,
                    scale=1.0 / (B - 1),
                )
            cc_in = dram.tile([1, 2 * F], FP16)
            cc_out = dram.tile([1, 2 * F], FP16)
            nc.sync.dma_start(out=cc_in[:], in_=cc_stage[:])
            nc.gpsimd.collective_compute(
                "AllReduce",
                ALU.add,
                replica_groups=groups,
                ins=[cc_in[:].opt()],
                outs=[cc_out[:].opt()],
            )

            # Global mean broadcast (the only thing pass2 deltas wait on).
            gm16 = stats.tile([P, F], FP16)
            nc.sync.dma_start(
                out=gm16[:], in_=cc_out[:, 0:F].to_broadcast([P, F])
            )

            # ---- rstd chain, packed [128, 8] (f = p*8 + j), fp32 work ----
            s12p = stats.tile([P, 2, 8], FP16)
            nc.sync.dma_start(
                out=s12p[:],
                in_=cc_out[:].rearrange("a (h p j) -> a p h j", h=2, p=P, j=8),
            )
            a1 = s12p[:, 0, :]  # global mean (fp16)
            a2 = s12p[:, 1, :]  # sum(x^2)/(N-1) (fp16)
            finw = stats.tile([P, 32], FP32)
            w1, w2, w3, w4 = (finw[:, 8 * i : 8 * (i + 1)] for i in range(4))
            nc.vector.tensor_tensor(w1, a1, a1, ALU.mult)  # mean^2
            # var = a2 - mean^2 * N/(N-1) + S_in/(N-1)
            nc.vector.scalar_tensor_tensor(
                w2, w1, -float(B) / (B - 1), a2, ALU.mult, ALU.add
            )
            nc.vector.scalar_tensor_tensor(
                w2, sinp[:], 1.0 / (B - 1), w2, ALU.mult, ALU.add
            )
            nc.scalar.activation(w3, w2, AF.Sqrt)
            nc.scalar.activation(w4, w3, AF.Copy, bias=EPS)
            rinv = stats.tile([P, 8], FP32)
            nc.vector.reciprocal(rinv, w4)
            r16p = stats.tile([P, 8], FP16)
            nc.scalar.copy(r16p[:], rinv[:])
            rd = dram.tile([1, F], FP16)
            nc.sync.dma_start(
                out=rd[:].rearrange("a (p j) -> a p j", p=P, j=8), in_=r16p[:]
            )
            rb16 = stats.tile([P, F], FP16)
            nc.sync.dma_start(out=rb16[:], in_=rd[:].to_broadcast([P, F]))

            # ---- Phase C: per chunk, all-fp16 on DVE ----
            db16 = [stats.tile([P, F], FP16, name=f"db16_{c}") for c in range(NCHUNK)]
            for c in range(NCHUNK):
                # delta_c = global_mean - m~_c  (both fp16, already broadcast)
                nc.vector.tensor_tensor(db16[c][:], gm16, mb16[c], ALU.subtract)
                ysl = y16[:, c * TPC : (c + 1) * TPC, :]
                db = db16[c][:, None, :].to_broadcast([P, TPC, F])
                rb = rb16[:, None, :].to_broadcast([P, TPC, F])
                nc.vector.tensor_tensor(ysl, ysl, db, ALU.subtract)
                nc.vector.tensor_tensor(ysl, ysl, rb, ALU.mult)
                for j in range(TPC):
                    t = c * TPC + j
                    nc.sync.dma_start(out=out_t[t], in_=y16[:, t, :])

    nc.finalize()
    return nc


@functools.cache
def _get_nc():
    return build_kernel()


def kernel(x, M, S, _trace=False, _trace_kwargs=None):
    del M  # overwritten by the first Welford step in the reference
    x = np.ascontiguousarray(x, dtype=np.float32)
    S = np.ascontiguousarray(S, dtype=np.float32).reshape(1, F)
    nc = _get_nc()
    in_maps = [
        {"x": x[i * ROWS : (i + 1) * ROWS], "S": S} for i in range(NCORES)
    ]
    res = run_bass_kernel_spmd(
        nc,
        in_maps,
        core_ids=list(range(NCORES)),
        trace=_trace,
        **(_trace_kwargs or {}),
    )
    out = np.concatenate(
        [res.results[i]["out"] for i in range(NCORES)], axis=0
    ).astype(np.float32)
    if _trace:
        return out, res
    return out


# revision 8
# speedup vs baseline: 1.1742x; 1.0925x over previous
"""MeanStdFilter kernel for 8 Trainium2 NeuronCores (v3).

Semantics (matches the sequential-Welford reference with M=0, S=S_in, n=0):
    S1[f] = sum_b x[b, f]            (global, over all 32768 rows)
    S2[f] = sum_b x[b, f]^2
    mean  = S1 / N
    var   = (S2 - S1^2/N + S_in) / (N - 1)
    out   = (x - mean) / (sqrt(var) + 1e-5)
The input running-mean buffer M is overwritten by the first Welford step in
the reference, so it never affects the output.

v3 architecture (vs the 200.5us v2 baseline):
  - Stats entirely on PE: per-chunk S1 via float32r ones-matmuls (1 cyc/row
    at free>=256 -> ~4x faster than fp32), S2 via fp16 ones-matmuls over
    Act-engine squares. DVE does no stats work.
  - Pre-AllReduce normalize pass1: y16 = x - m~_c (per-chunk mean broadcast)
    runs on DVE *during* the load phase, hidden under HBM traffic. The
    fp32 -> fp16 rounding happens after the subtract, so output error stays
    multiplicative in |out| (no blow-up of relative error near zero).
  - 4KB fp16 AllReduce payload, prescaled so cols 0:1024 are the global
    mean directly; the chunk totals are summed on DVE ([1,1024] adds).
  - Post-AR critical path: broadcast global mean -> delta_c = gmean - m~_c
    (all-fp16 DVE), then per chunk: y -= delta_c; y *= rstd (all-fp16 DVE
    2x mode), fp16 stores (half the write traffic; host upcasts).
  - rstd chain runs packed [128,8] off the critical path, concurrent with
    the first pass2 chunk. Sqrt activation table preloaded during phase A.
  - Warmup AllReduce kept: it pulls the one-time CC-init/skew barrier
    (~35us) into the load phase.
"""

import functools

import numpy as np

import concourse.bacc as bacc
import concourse.tile as tile
from concourse import mybir
from concourse.bass_utils import run_bass_kernel_spmd

NCORES = 8
B, F = 32768, 1024
ROWS = B // NCORES  # 4096 rows per core
P = 128
NT = ROWS // P  # 32 row-tiles of [128, 1024] per core
TPC = 8  # tiles per chunk
NCHUNK = NT // TPC  # 4
CROWS = P * TPC  # 1024 rows per chunk
EPS = 1e-5
FP32 = mybir.dt.float32
FP32R = mybir.dt.float32r
FP16 = mybir.dt.float16
AF = mybir.ActivationFunctionType
ALU = mybir.AluOpType


def build_kernel():
    nc = bacc.Bacc(
        "TRN2", target_bir_lowering=False, debug=False, num_devices=NCORES
    )
    x = nc.declare_dram_parameter("x", [ROWS, F], FP32R, isOutput=False)
    s_in = nc.declare_dram_parameter("S", [1, F], FP32, isOutput=False)
    out = nc.declare_dram_parameter("out", [ROWS, F], FP16, isOutput=True)

    x_t = x[:].rearrange("(n p) f -> n p f", p=P)
    out_t = out[:].rearrange("(n p) f -> n p f", p=P)
    groups = [list(range(NCORES))]

    with tile.TileContext(nc) as tc:
        with (
            tc.tile_pool(name="xst", bufs=2) as xst_pool,
            tc.tile_pool(name="ybuf", bufs=1) as ybuf,
            tc.tile_pool(name="sq", bufs=3) as sqpool,
            tc.tile_pool(name="stats", bufs=1) as stats,
            tc.tile_pool(name="psum", bufs=1, space="PSUM") as psum,
            tc.tile_pool(name="dram", bufs=1, space="DRAM") as dram,
        ):
            # Warmup AllReduce: pulls the one-time CC-init / start-skew
            # barrier into the load phase. Result is unused.
            wu = stats.tile([1, 8], FP32)
            nc.vector.memset(wu, 0.0)
            wu_in = dram.tile([1, 8], FP32)
            wu_out = dram.tile([1, 8], FP32)
            nc.sync.dma_start(out=wu_in[:], in_=wu[:])
            nc.gpsimd.collective_compute(
                "AllReduce",
                ALU.add,
                replica_groups=groups,
                ins=[wu_in[:].opt()],
                outs=[wu_out[:].opt()],
            )

            ones = stats.tile([P, 1], FP32)
            nc.vector.memset(ones, 1.0)
            ones16 = stats.tile([P, 1], FP16)
            nc.vector.memset(ones16, 1.0)
            # float32r weights must come from a rounding producer (Act copy),
            # not a bitcast — the BIR verifier enforces this.
            ones_r = stats.tile([P, 1], FP32R)
            nc.scalar.copy(ones_r[:], ones[:])

            # Preload the Sqrt activation table so the post-AR rstd chain
            # doesn't pay the ~1.3us table swap on the critical path.
            dummy = stats.tile([1, 8], FP32)
            nc.scalar.activation(dummy, wu[:], AF.Sqrt)

            # S buffer, packed per-feature layout f = p*8 + j.
            sinp = stats.tile([P, 8], FP32)
            nc.sync.dma_start(
                out=sinp[:], in_=s_in[:].rearrange("a (p j) -> a p j", p=P, j=8)
            )

            # Resident normalized-intermediate shard (fp16), 64KB/partition.
            y16 = ybuf.tile([P, NT, F], FP16, name="y16")

            # Per-chunk S1 PSUM pair (reused across the 4 chunk groups) and
            # global S2 pair (one accumulation group over all 32 tiles).
            ps1 = [psum.tile([1, 512], FP32, tag=f"ps1_{h}", name=f"ps1_{h}") for h in range(2)]
            ps2 = [psum.tile([1, 512], FP32, tag=f"ps2_{h}", name=f"ps2_{h}") for h in range(2)]

            # Chunk mean staging: fp16 row, DRAM bounce, [128,F] broadcast.
            m16 = [stats.tile([1, F], FP16, name=f"m16_{c}") for c in range(NCHUNK)]
            s_c = [stats.tile([1, F], FP32, name=f"s_c{c}") for c in range(NCHUNK)]
            s_tot = stats.tile([1, F], FP32)
            md = dram.tile([NCHUNK, F], FP16)
            mb16 = [
                stats.tile([P, F], FP16, name=f"mb16_{c}") for c in range(NCHUNK)
            ]

            # ---- Phase A: stream chunks, stats on PE/Act, pass1 on DVE ----
            for c in range(NCHUNK):
                xc = xst_pool.tile([P, TPC, F], FP32R, tag="xst", name=f"x_c{c}")
                for j in range(TPC):
                    t = c * TPC + j
                    xt = xc[:, j, :]
                    nc.sync.dma_start(out=xt, in_=x_t[t])
                    sq = sqpool.tile([P, F], FP16, tag="sq")
                    nc.scalar.activation(sq, xt.bitcast(FP32), AF.Square)
                    for h in range(2):
                        hs = slice(h * 512, (h + 1) * 512)
                        nc.tensor.matmul(
                            ps1[h][:],
                            lhsT=ones_r[:],
                            rhs=xt[:, hs],
                            start=(j == 0),
                            stop=(j == TPC - 1),
                        )
                        nc.tensor.matmul(
                            ps2[h][:],
                            lhsT=ones16[:],
                            rhs=sq[:, hs],
                            start=(t == 0),
                            stop=(t == NT - 1),
                        )
                # Chunk stats: raw sums (for the AR payload) and the fp16
                # chunk mean m~_c (for pass1).
                for h in range(2):
                    hs = slice(h * 512, (h + 1) * 512)
                    nc.scalar.copy(s_c[c][:, hs], ps1[h][:])
                    nc.scalar.activation(
                        m16[c][:, hs], ps1[h][:], AF.Copy, scale=1.0 / CROWS
                    )
                # Accumulate the running total on DVE ahead of pass1 so the
                # AllReduce payload never waits on pass1 progress.
                if c == 0:
                    nc.vector.tensor_copy(s_tot[:], s_c[c])
                else:
                    nc.vector.tensor_tensor(s_tot[:], s_tot, s_c[c], ALU.add)
                # Broadcast m~_c to all partitions via a DRAM bounce.
                nc.sync.dma_start(out=md[c : c + 1, :], in_=m16[c][:])
                nc.sync.dma_start(
                    out=mb16[c][:], in_=md[c : c + 1, :].to_broadcast([P, F])
                )
                # pass1: y16 = x - m~_c  (fp32 math, fp16 result: the error
                # stays proportional to |x - m~_c|).
                mb = mb16[c][:, None, :].to_broadcast([P, TPC, F])
                nc.vector.tensor_tensor(
                    y16[:, c * TPC : (c + 1) * TPC, :],
                    xc[:].bitcast(FP32),
                    mb,
                    ALU.subtract,
                )

            # ---- AllReduce: [mean | S2/(N-1)] in fp16, 4KB ----
            cc_stage = stats.tile([1, 2 * F], FP16)
            nc.scalar.activation(
                cc_stage[:, 0:F], s_tot[:], AF.Copy, scale=1.0 / B
            )
            for h in range(2):
                nc.scalar.activation(
                    cc_stage[:, F + h * 512 : F + (h + 1) * 512],
                    ps2[h][:],
                    AF.Copy,
                    scale=1.0 / (B - 1),
                )
            cc_in = dram.tile([1, 2 * F], FP16)
            cc_out = dram.tile([1, 2 * F], FP16)
            nc.sync.dma_start(out=cc_in[:], in_=cc_stage[:])
            nc.gpsimd.collective_compute(
                "AllReduce",
                ALU.add,
                replica_groups=groups,
                ins=[cc_in[:].opt()],
                outs=[cc_out[:].opt()],
            )

            # Global mean broadcast (the only thing pass2 deltas wait on).
            gm16 = stats.tile([P, F], FP16)
            nc.sync.dma_start(
                out=gm16[:], in_=cc_out[:, 0:F].to_broadcast([P, F])
            )

            # ---- rstd chain, packed [128, 8] (f = p*8 + j), fp32 work ----
            s12p = stats.tile([P, 2, 8], FP16)
            nc.sync.dma_start(
                out=s12p[:],
                in_=cc_out[:].rearrange("a (h p j) -> a p h j", h=2, p=P, j=8),
            )
            a1 = s12p[:, 0, :]  # global mean (fp16)
            a2 = s12p[:, 1, :]  # sum(x^2)/(N-1) (fp16)
            finw = stats.tile([P, 32], FP32)
            w1, w2, w3, w4 = (finw[:, 8 * i : 8 * (i + 1)] for i in range(4))
            nc.vector.tensor_tensor(w1, a1, a1, ALU.mult)  # mean^2
            # var = a2 - mean^2 * N/(N-1) + S_in/(N-1)
            nc.vector.scalar_tensor_tensor(
                w2, w1, -float(B) / (B - 1), a2, ALU.mult, ALU.add
            )
            nc.vector.scalar_tensor_tensor(
                w2, sinp[:], 1.0 / (B - 1), w2, ALU.mult, ALU.add
            )
            nc.scalar.activation(w3, w2, AF.Sqrt)
            nc.scalar.activation(w4, w3, AF.Copy, bias=EPS)
            rinv = stats.tile([P, 8], FP32)
            nc.vector.reciprocal(rinv, w4)
            r16p = stats.tile([P, 8], FP16)
            nc.scalar.copy(r16p[:], rinv[:])
            rd = dram.tile([1, F], FP16)
            nc.sync.dma_start(
                out=rd[:].rearrange("a (p j) -> a p j", p=P, j=8), in_=r16p[:]
            )
            rb16 = stats.tile([P, F], FP16)
            nc.sync.dma_start(out=rb16[:], in_=rd[:].to_broadcast([P, F]))

            # ---- Phase C: per chunk, all-fp16 on DVE ----
            db16 = [stats.tile([P, F], FP16, name=f"db16_{c}") for c in range(NCHUNK)]
            HC = TPC // 2  # half-chunk pieces: stores start after ~2.2us ops
            for c in range(NCHUNK):
                # delta_c = global_mean - m~_c  (both fp16, already broadcast)
                nc.vector.tensor_tensor(db16[c][:], gm16, mb16[c], ALU.subtract)
                db = db16[c][:, None, :].to_broadcast([P, HC, F])
                rb = rb16[:, None, :].to_broadcast([P, HC, F])
                for p in range(2):
                    t0 = c * TPC + p * HC
                    ysl = y16[:, t0 : t0 + HC, :]
                    nc.vector.tensor_tensor(ysl, ysl, db, ALU.subtract)
                    nc.vector.tensor_tensor(ysl, ysl, rb, ALU.mult)
                    for j in range(HC):
                        t = t0 + j
                        nc.sync.dma_start(out=out_t[t], in_=y16[:, t, :])

    nc.finalize()
    return nc


@functools.cache
def _get_nc():
    return build_kernel()


def kernel(x, M, S, _trace=False, _trace_kwargs=None):
    del M  # overwritten by the first Welford step in the reference
    x = np.ascontiguousarray(x, dtype=np.float32)
    S = np.ascontiguousarray(S, dtype=np.float32).reshape(1, F)
    nc = _get_nc()
    in_maps = [
        {"x": x[i * ROWS : (i + 1) * ROWS], "S": S} for i in range(NCORES)
    ]
    res = run_bass_kernel_spmd(
        nc,
        in_maps,
        core_ids=list(range(NCORES)),
        trace=_trace,
        **(_trace_kwargs or {}),
    )
    out = np.concatenate(
        [res.results[i]["out"] for i in range(NCORES)], axis=0
    ).astype(np.float32)
    if _trace:
        return out, res
    return out


# revision 10
# speedup vs baseline: 1.2637x; 1.0762x over previous
"""MeanStdFilter kernel for 8 Trainium2 NeuronCores (v5).

Semantics (matches the sequential-Welford reference with M=0, S=S_in, n=0):
    S1[f] = sum_b x[b, f]            (global, over all 32768 rows)
    S2[f] = sum_b x[b, f]^2
    mean  = S1 / N
    var   = (S2 - S1^2/N + S_in) / (N - 1)
    out   = (x - mean) / (sqrt(var) + 1e-5)
The input running-mean buffer M is overwritten by the first Welford step in
the reference, so it never affects the output.

Architecture (baseline 200.5us -> v4 170.8us -> this):
  - x is loaded as float32r so the per-chunk S1 ones-matmuls run at the
    fast PE rate; S2 runs as fp16 matmuls over Act-engine squares. DVE
    does no stats reduction work.
  - Pre-AllReduce normalize pass1: y16 = x - m~_c (chunk mean broadcast)
    runs on DVE during the load phase, hidden under HBM traffic. The
    fp32 -> fp16 rounding happens after the subtract, so output error
    stays multiplicative in |out| (no relative-error blowup near zero).
  - Per-chunk S1 PSUM pairs ping-pong so the next chunk's matmuls never
    wait on the previous chunk's snapshot readers. S1 chunk totals are
    accumulated straight from PSUM on DVE.
  - 4KB fp16 AllReduce payload, prescaled so cols 0:1024 are the global
    mean directly.
  - Post-AR: broadcast global mean -> delta_c = gmean - m~_c, then per
    half-chunk: y -= delta_c; y *= rstd (all-fp16 DVE 2x mode), fp16
    stores (half write traffic; host upcasts). rstd chain runs packed
    [128,8] concurrently with the first pass2a pieces.
  - Warmup AllReduce on an *uninitialized* scratch buffer is issued
    before anything else: it pulls the one-time CC-init/skew barrier
    into the load phase without delaying the first x-tile DMAs.
"""

import functools

import numpy as np

import concourse.bacc as bacc
import concourse.tile as tile
from concourse import mybir
from concourse.bass_utils import run_bass_kernel_spmd

NCORES = 8
B, F = 32768, 1024
ROWS = B // NCORES  # 4096 rows per core
P = 128
NT = ROWS // P  # 32 row-tiles of [128, 1024] per core
TPC = 8  # tiles per chunk
NCHUNK = NT // TPC  # 4
CROWS = P * TPC  # 1024 rows per chunk
EPS = 1e-5
FP32 = mybir.dt.float32
FP32R = mybir.dt.float32r
FP16 = mybir.dt.float16
AF = mybir.ActivationFunctionType
ALU = mybir.AluOpType


def build_kernel():
    nc = bacc.Bacc(
        "TRN2", target_bir_lowering=False, debug=False, num_devices=NCORES
    )
    x = nc.declare_dram_parameter("x", [ROWS, F], FP32R, isOutput=False)
    s_in = nc.declare_dram_parameter("S", [1, F], FP32, isOutput=False)
    out = nc.declare_dram_parameter("out", [ROWS, F], FP16, isOutput=True)

    x_t = x[:].rearrange("(n p) f -> n p f", p=P)
    out_t = out[:].rearrange("(n p) f -> n p f", p=P)
    groups = [list(range(NCORES))]

    with tile.TileContext(nc) as tc:
        with (
            tc.tile_pool(name="xst", bufs=3) as xst_pool,
            tc.tile_pool(name="ybuf", bufs=1) as ybuf,
            tc.tile_pool(name="sq", bufs=3) as sqpool,
            tc.tile_pool(name="db", bufs=2) as dbpool,
            tc.tile_pool(name="stats", bufs=1) as stats,
            tc.tile_pool(name="psum", bufs=1, space="PSUM") as psum,
            tc.tile_pool(name="dram", bufs=1, space="DRAM") as dram,
        ):
            # Warmup AllReduce on garbage DRAM: pulls the one-time CC-init /
            # start-skew barrier into the load phase. Issued first so no
            # engine work delays it; result and input are never read.
            wu_in = dram.tile([1, 8], FP32)
            wu_out = dram.tile([1, 8], FP32)
            nc.gpsimd.collective_compute(
                "AllReduce",
                ALU.add,
                replica_groups=groups,
                ins=[wu_in[:].opt()],
                outs=[wu_out[:].opt()],
            )

            ones16 = stats.tile([P, 1], FP16)
            nc.vector.memset(ones16, 1.0)
            ones_r = stats.tile([P, 1], FP32R)
            nc.scalar.activation(
                ones_r[:], ones16[:], AF.Copy
            )  # fp32r needs a rounding producer

            # Resident normalized-intermediate shard (fp16), 64KB/partition.
            y16 = ybuf.tile([P, NT, F], FP16, name="y16")

            # Per-chunk S1 PSUM pairs (ping-pong) + global S2 pair.
            ps1 = [
                [
                    psum.tile([1, 512], FP32, tag=f"ps1_{k}_{h}", name=f"ps1_{k}_{h}")
                    for h in range(2)
                ]
                for k in range(2)
            ]
            ps2 = [
                psum.tile([1, 512], FP32, tag=f"ps2_{h}", name=f"ps2_{h}")
                for h in range(2)
            ]

            s_tot = stats.tile([1, F], FP32)
            m16 = [stats.tile([1, F], FP16, name=f"m16_{c}") for c in range(NCHUNK)]
            md = dram.tile([NCHUNK, F], FP16)
            mb16 = [
                stats.tile([P, F], FP16, name=f"mb16_{c}") for c in range(NCHUNK)
            ]
            sinp = stats.tile([P, 8], FP32)
            dummy = stats.tile([1, 8], FP16)

            # ---- Phase A: stream chunks; stats on PE/Act, pass1 on DVE ----
            for c in range(NCHUNK):
                xc = xst_pool.tile([P, TPC, F], FP32R, tag="xst", name=f"x_c{c}")
                p1 = ps1[c % 2]
                for j in range(TPC):
                    t = c * TPC + j
                    xt = xc[:, j, :]
                    nc.sync.dma_start(out=xt, in_=x_t[t])
                    sq = sqpool.tile([P, F], FP16, tag="sq")
                    nc.scalar.activation(sq, xt.bitcast(FP32), AF.Square)
                    for h in range(2):
                        hs = slice(h * 512, (h + 1) * 512)
                        nc.tensor.matmul(
                            p1[h][:],
                            lhsT=ones_r[:],
                            rhs=xt[:, hs],
                            start=(j == 0),
                            stop=(j == TPC - 1),
                        )
                        nc.tensor.matmul(
                            ps2[h][:],
                            lhsT=ones16[:],
                            rhs=sq[:, hs],
                            start=(t == 0),
                            stop=(t == NT - 1),
                        )
                if c == 0:
                    # Deferred setup: issued after chunk 0's loads so the
                    # x DMAs are first in the queues. Sqrt table preload
                    # rides the Act slack mid-phase.
                    nc.sync.dma_start(
                        out=sinp[:],
                        in_=s_in[:].rearrange("a (p j) -> a p j", p=P, j=8),
                    )
                    nc.scalar.activation(dummy, sinp[0:1, :], AF.Sqrt)
                # Chunk stats: running S1 total (DVE, straight from PSUM)
                # and the fp16 chunk mean m~_c (Act) for pass1.
                for h in range(2):
                    hs = slice(h * 512, (h + 1) * 512)
                    if c == 0:
                        nc.vector.tensor_copy(s_tot[:, hs], p1[h])
                    else:
                        nc.vector.tensor_tensor(
                            s_tot[:, hs], s_tot[:, hs], p1[h], ALU.add
                        )
                    nc.scalar.activation(
                        m16[c][:, hs], p1[h][:], AF.Copy, scale=1.0 / CROWS
                    )
                # Broadcast m~_c to all partitions via a DRAM bounce.
                nc.sync.dma_start(out=md[c : c + 1, :], in_=m16[c][:])
                nc.sync.dma_start(
                    out=mb16[c][:], in_=md[c : c + 1, :].to_broadcast([P, F])
                )
                # pass1: y16 = x - m~_c  (fp32 math, fp16 result: the error
                # stays proportional to |x - m~_c|).
                mb = mb16[c][:, None, :].to_broadcast([P, TPC, F])
                nc.vector.tensor_tensor(
                    y16[:, c * TPC : (c + 1) * TPC, :],
                    xc[:].bitcast(FP32),
                    mb,
                    ALU.subtract,
                )

            # ---- AllReduce: [mean | S2/(N-1)] in fp16, 4KB ----
            cc_stage = stats.tile([1, 2 * F], FP16)
            nc.scalar.activation(
                cc_stage[:, 0:F], s_tot[:], AF.Copy, scale=1.0 / B
            )
            for h in range(2):
                nc.scalar.activation(
                    cc_stage[:, F + h * 512 : F + (h + 1) * 512],
                    ps2[h][:],
                    AF.Copy,
                    scale=1.0 / (B - 1),
                )
            cc_in = dram.tile([1, 2 * F], FP16)
            cc_out = dram.tile([1, 2 * F], FP16)
            nc.sync.dma_start(out=cc_in[:], in_=cc_stage[:])
            nc.gpsimd.collective_compute(
                "AllReduce",
                ALU.add,
                replica_groups=groups,
                ins=[cc_in[:].opt()],
                outs=[cc_out[:].opt()],
            )

            # Global mean broadcast (gates the pass2 deltas).
            gm16 = stats.tile([P, F], FP16)
            nc.sync.dma_start(
                out=gm16[:], in_=cc_out[:, 0:F].to_broadcast([P, F])
            )

            # ---- rstd chain, packed [128, 8] (f = p*8 + j), fp32 work ----
            s12p = stats.tile([P, 2, 8], FP16)
            nc.sync.dma_start(
                out=s12p[:],
                in_=cc_out[:].rearrange("a (h p j) -> a p h j", h=2, p=P, j=8),
            )
            a1 = s12p[:, 0, :]  # global mean (fp16)
            a2 = s12p[:, 1, :]  # sum(x^2)/(N-1) (fp16)
            finw = stats.tile([P, 32], FP32)
            w1, w2, w3, w4 = (finw[:, 8 * i : 8 * (i + 1)] for i in range(4))
            nc.vector.tensor_tensor(w1, a1, a1, ALU.mult)  # mean^2
            # var = a2 - mean^2 * N/(N-1) + S_in/(N-1)
            nc.vector.scalar_tensor_tensor(
                w2, w1, -float(B) / (B - 1), a2, ALU.mult, ALU.add
            )
            nc.vector.scalar_tensor_tensor(
                w2, sinp[:], 1.0 / (B - 1), w2, ALU.mult, ALU.add
            )
            nc.scalar.activation(w3, w2, AF.Sqrt)
            nc.scalar.activation(w4, w3, AF.Copy, bias=EPS)
            rinv = stats.tile([P, 8], FP32)
            nc.vector.reciprocal(rinv, w4)
            r16p = stats.tile([P, 8], FP16)
            nc.scalar.copy(r16p[:], rinv[:])
            rd = dram.tile([1, F], FP16)
            nc.sync.dma_start(
                out=rd[:].rearrange("a (p j) -> a p j", p=P, j=8), in_=r16p[:]
            )
            rb16 = stats.tile([P, F], FP16)
            nc.sync.dma_start(out=rb16[:], in_=rd[:].to_broadcast([P, F]))

            # ---- Phase C: per half-chunk, all-fp16 on DVE ----
            HC = TPC // 2
            for c in range(NCHUNK):
                # delta_c = global_mean - m~_c  (both fp16, already broadcast)
                db16 = dbpool.tile([P, F], FP16, tag="db")
                nc.vector.tensor_tensor(db16[:], gm16, mb16[c], ALU.subtract)
                db = db16[:, None, :].to_broadcast([P, HC, F])
                rb = rb16[:, None, :].to_broadcast([P, HC, F])
                for p in range(2):
                    t0 = c * TPC + p * HC
                    ysl = y16[:, t0 : t0 + HC, :]
                    nc.vector.tensor_tensor(ysl, ysl, db, ALU.subtract)
                    nc.vector.tensor_tensor(ysl, ysl, rb, ALU.mult)
                    for j in range(HC):
                        t = t0 + j
                        nc.sync.dma_start(out=out_t[t], in_=y16[:, t, :])

    nc.finalize()
    return nc


@functools.cache
def _get_nc():
    return build_kernel()


def kernel(x, M, S, _trace=False, _trace_kwargs=None):
    del M  # overwritten by the first Welford step in the reference
    x = np.ascontiguousarray(x, dtype=np.float32)
    S = np.ascontiguousarray(S, dtype=np.float32).reshape(1, F)
    nc = _get_nc()
    in_maps = [
        {"x": x[i * ROWS : (i + 1) * ROWS], "S": S} for i in range(NCORES)
    ]
    res = run_bass_kernel_spmd(
        nc,
        in_maps,
        core_ids=list(range(NCORES)),
        trace=_trace,
        **(_trace_kwargs or {}),
    )
    out = np.concatenate(
        [res.results[i]["out"] for i in range(NCORES)], axis=0
    ).astype(np.float32)
    if _trace:
        return out, res
    return out


# revision 16
# speedup vs baseline: 1.2726x; 1.0070x over previous
"""MeanStdFilter kernel for 8 Trainium2 NeuronCores (v6).

Semantics (matches the sequential-Welford reference with M=0, S=S_in, n=0):
    S1[f] = sum_b x[b, f]            (global, over all 32768 rows)
    S2[f] = sum_b x[b, f]^2
    mean  = S1 / N
    var   = (S2 - S1^2/N + S_in) / (N - 1)
    out   = (x - mean) / (sqrt(var) + 1e-5)
The input running-mean buffer M is overwritten by the first Welford step in
the reference, so it never affects the output.

Architecture (baseline 200.5us -> v5 158.7us -> this):
  - x loaded as float32r; per-chunk S1 ones-matmuls + global fp16 S2
    matmuls over Act squares, same-weight matmuls grouped in pairs so
    LDWEIGHTS alternation doesn't break the PE pipeline.
  - All [128,F] per-feature broadcasts (chunk mean, global mean) are PE
    outer-products (ones_row x row) into PSUM — no DRAM bounce, no
    queueing behind bulk loads.
  - Pre-AllReduce pass1 on DVE: y16 = x - m~_c (m~_c from PSUM), hidden
    under the load phase. fp32 math, fp16 result: output error stays
    multiplicative in |out|.
  - S1 chunk totals accumulated on the otherwise-idle GpSimd engine so
    neither PE (bank reuse anti-dep) nor DVE (busy with pass1) stalls.
  - 4KB fp16 AllReduce payload, prescaled so cols 0:1024 are the global
    mean directly. Warmup AllReduce on uninitialized scratch issued
    first pulls the one-time CC-init/skew barrier into the load phase.
  - Post-AR: delta_c = gm_psum - m~_c_psum (PE outer-products, DVE
    subtract), then per half-chunk: y -= delta_c; y *= rstd (all-fp16
    DVE 2x mode), fp16 stores (half write traffic; host upcasts). rstd
    chain runs packed [128,8] concurrently with the first pass2 pieces.
"""

import functools

import numpy as np

import concourse.bacc as bacc
import concourse.tile as tile
from concourse import mybir
from concourse.bass_utils import run_bass_kernel_spmd

NCORES = 8
B, F = 32768, 1024
ROWS = B // NCORES  # 4096 rows per core
P = 128
NT = ROWS // P  # 32 row-tiles of [128, 1024] per core
TPC = 8  # tiles per chunk
NCHUNK = NT // TPC  # 4
CROWS = P * TPC  # 1024 rows per chunk
EPS = 1e-5
FP32 = mybir.dt.float32
FP32R = mybir.dt.float32r
FP16 = mybir.dt.float16
AF = mybir.ActivationFunctionType
ALU = mybir.AluOpType


def build_kernel():
    nc = bacc.Bacc(
        "TRN2", target_bir_lowering=False, debug=False, num_devices=NCORES
    )
    x = nc.declare_dram_parameter("x", [ROWS, F], FP32R, isOutput=False)
    s_in = nc.declare_dram_parameter("S", [1, F], FP32, isOutput=False)
    out = nc.declare_dram_parameter("out", [ROWS, F], FP16, isOutput=True)

    x_t = x[:].rearrange("(n p) f -> n p f", p=P)
    out_t = out[:].rearrange("(n p) f -> n p f", p=P)
    groups = [list(range(NCORES))]

    with tile.TileContext(nc) as tc:
        with (
            tc.tile_pool(name="xst", bufs=3) as xst_pool,
            tc.tile_pool(name="ybuf", bufs=1) as ybuf,
            tc.tile_pool(name="sq", bufs=3) as sqpool,
            tc.tile_pool(name="db", bufs=2) as dbpool,
            tc.tile_pool(name="stats", bufs=1) as stats,
            tc.tile_pool(name="psum", bufs=1, space="PSUM") as psum,
            tc.tile_pool(name="dram", bufs=1, space="DRAM") as dram,
        ):
            # Warmup AllReduce on garbage DRAM: pulls the one-time CC-init /
            # start-skew barrier into the load phase. Input is never read.
            wu_in = dram.tile([1, 8], FP32)
            wu_out = dram.tile([1, 8], FP32)
            nc.gpsimd.collective_compute(
                "AllReduce",
                ALU.add,
                replica_groups=groups,
                ins=[wu_in[:].opt()],
                outs=[wu_out[:].opt()],
            )

            ones16 = stats.tile([P, 1], FP16)
            nc.vector.memset(ones16, 1.0)
            ones_row = stats.tile([1, P], FP16)  # outer-product weights
            nc.vector.memset(ones_row, 1.0)
            ones_r = stats.tile([P, 1], FP32R)
            nc.scalar.activation(
                ones_r[:], ones16[:], AF.Copy
            )  # fp32r needs a rounding producer

            # Resident normalized-intermediate shard (fp16), 64KB/partition.
            y16 = ybuf.tile([P, NT, F], FP16, name="y16")

            # PSUM: S1 chunk pair, S2 global pair, chunk-mean broadcast
            # pair, global-mean broadcast pair -> 4 small + 4 full banks.
            ps1 = [
                psum.tile([1, 512], FP32, tag=f"ps1_{h}", name=f"ps1_{h}")
                for h in range(2)
            ]
            ps2 = [
                psum.tile([1, 512], FP32, tag=f"ps2_{h}", name=f"ps2_{h}")
                for h in range(2)
            ]
            mcb = [
                psum.tile([P, 512], FP32, tag=f"mcb_{h}", name=f"mcb_{h}")
                for h in range(2)
            ]
            gmp = [
                psum.tile([P, 512], FP32, tag=f"gmp_{h}", name=f"gmp_{h}")
                for h in range(2)
            ]

            s_tot = stats.tile([1, F], FP32)
            m16 = [stats.tile([1, F], FP16, name=f"m16_{c}") for c in range(NCHUNK)]
            sinp = stats.tile([P, 8], FP32)
            dummy = stats.tile([1, 8], FP16)

            # ---- Phase A: stream chunks; stats on PE/Act, pass1 on DVE ----
            for c in range(NCHUNK):
                xc = xst_pool.tile([P, TPC, F], FP32R, tag="xst", name=f"x_c{c}")
                for j in range(TPC):
                    t = c * TPC + j
                    xt = xc[:, j, :]
                    nc.sync.dma_start(out=xt, in_=x_t[t])
                    sq = sqpool.tile([P, F], FP16, tag="sq")
                    nc.scalar.activation(sq, xt.bitcast(FP32), AF.Square)
                    for h in range(2):
                        nc.tensor.matmul(
                            ps1[h][:],
                            lhsT=ones_r[:],
                            rhs=xt[:, h * 512 : (h + 1) * 512],
                            start=(j == 0),
                            stop=(j == TPC - 1),
                        )
                    for h in range(2):
                        nc.tensor.matmul(
                            ps2[h][:],
                            lhsT=ones16[:],
                            rhs=sq[:, h * 512 : (h + 1) * 512],
                            start=(t == 0),
                            stop=(t == NT - 1),
                        )
                # Chunk stats: fp16 chunk mean m~_c (Act), running S1 total
                # (GpSimd, straight from PSUM — keeps PE/DVE unblocked),
                # then broadcast m~_c to 128 partitions via PE outer-product.
                for h in range(2):
                    hs = slice(h * 512, (h + 1) * 512)
                    nc.scalar.activation(
                        m16[c][:, hs], ps1[h][:], AF.Copy, scale=1.0 / CROWS
                    )
                # GpSimd can't read PSUM: accumulate the sum of fp16 chunk
                # means instead (costs ~2e-6 absolute on the global mean);
                # the CROWS/B scale is folded into the AR staging copy.
                if c == 0:
                    nc.gpsimd.tensor_copy(s_tot[:], m16[c])
                else:
                    nc.gpsimd.tensor_tensor(s_tot[:], s_tot, m16[c], ALU.add)
                for h in range(2):
                    nc.tensor.matmul(
                        mcb[h][:],
                        lhsT=ones_row[:],
                        rhs=m16[c][:, h * 512 : (h + 1) * 512],
                        start=True,
                        stop=True,
                    )
                # pass1: y16 = x - m~_c  (fp32 math, fp16 result), per half.
                for h in range(2):
                    hs = slice(h * 512, (h + 1) * 512)
                    nc.vector.tensor_tensor(
                        y16[:, c * TPC : (c + 1) * TPC, hs],
                        xc[:, :, hs].bitcast(FP32),
                        mcb[h][:, None, :].to_broadcast([P, TPC, 512]),
                        ALU.subtract,
                    )
                if c == 0:
                    # Deferred setup (after chunk 0 so x DMAs lead the
                    # queues; Sqrt table preload rides the Act slack).
                    nc.sync.dma_start(
                        out=sinp[:],
                        in_=s_in[:].rearrange("a (p j) -> a p j", p=P, j=8),
                    )
                    nc.scalar.activation(dummy, sinp[0:1, :], AF.Sqrt)

            # ---- AllReduce: [mean | S2/(N-1)] in fp16, 4KB ----
            cc_stage = stats.tile([1, 2 * F], FP16)
            nc.scalar.activation(
                cc_stage[:, 0:F], s_tot[:], AF.Copy, scale=float(CROWS) / B
            )
            for h in range(2):
                nc.scalar.activation(
                    cc_stage[:, F + h * 512 : F + (h + 1) * 512],
                    ps2[h][:],
                    AF.Copy,
                    scale=1.0 / (B - 1),
                )
            cc_in = dram.tile([1, 2 * F], FP16)
            cc_out = dram.tile([1, 2 * F], FP16)
            nc.sync.dma_start(out=cc_in[:], in_=cc_stage[:])
            nc.gpsimd.collective_compute(
                "AllReduce",
                ALU.add,
                replica_groups=groups,
                ins=[cc_in[:].opt()],
                outs=[cc_out[:].opt()],
            )

            # Global mean row -> PE outer-product broadcast -> SBUF fp16
            # (DVE may read at most one PSUM operand, so stage via Act).
            gm_row = stats.tile([1, F], FP16)
            nc.sync.dma_start(out=gm_row[:], in_=cc_out[:, 0:F])
            gm16 = stats.tile([P, F], FP16)
            for h in range(2):
                hs = slice(h * 512, (h + 1) * 512)
                nc.tensor.matmul(
                    gmp[h][:],
                    lhsT=ones_row[:],
                    rhs=gm_row[:, hs],
                    start=True,
                    stop=True,
                )
                nc.scalar.copy(gm16[:, hs], gmp[h][:])

            # ---- rstd chain, packed [128, 8] (f = p*8 + j), fp32 work ----
            s12p = stats.tile([P, 2, 8], FP16)
            nc.sync.dma_start(
                out=s12p[:],
                in_=cc_out[:].rearrange("a (h p j) -> a p h j", h=2, p=P, j=8),
            )
            a1 = s12p[:, 0, :]  # global mean (fp16)
            a2 = s12p[:, 1, :]  # sum(x^2)/(N-1) (fp16)
            finw = stats.tile([P, 32], FP32)
            w1, w2, w3, w4 = (finw[:, 8 * i : 8 * (i + 1)] for i in range(4))
            nc.vector.tensor_tensor(w1, a1, a1, ALU.mult)  # mean^2
            # var = a2 - mean^2 * N/(N-1) + S_in/(N-1)
            nc.vector.scalar_tensor_tensor(
                w2, w1, -float(B) / (B - 1), a2, ALU.mult, ALU.add
            )
            nc.vector.scalar_tensor_tensor(
                w2, sinp[:], 1.0 / (B - 1), w2, ALU.mult, ALU.add
            )
            nc.scalar.activation(w3, w2, AF.Sqrt)
            nc.scalar.activation(w4, w3, AF.Copy, bias=EPS)
            rinv = stats.tile([P, 8], FP32)
            nc.vector.reciprocal(rinv, w4)
            r16p = stats.tile([P, 8], FP16)
            nc.scalar.copy(r16p[:], rinv[:])
            rd = dram.tile([1, F], FP16)
            nc.sync.dma_start(
                out=rd[:].rearrange("a (p j) -> a p j", p=P, j=8), in_=r16p[:]
            )
            rb16 = stats.tile([P, F], FP16)
            nc.sync.dma_start(out=rb16[:], in_=rd[:].to_broadcast([P, F]))

            # ---- Phase C: per half-chunk, all-fp16 on DVE ----
            HC = TPC // 2
            for c in range(NCHUNK):
                # Recreate m~_c broadcast on PE (banks were recycled), then
                # delta_c = global_mean - m~_c in fp16.
                db16 = dbpool.tile([P, F], FP16, tag="db")
                for h in range(2):
                    hs = slice(h * 512, (h + 1) * 512)
                    nc.tensor.matmul(
                        mcb[h][:],
                        lhsT=ones_row[:],
                        rhs=m16[c][:, hs],
                        start=True,
                        stop=True,
                    )
                    nc.vector.tensor_tensor(
                        db16[:, hs], gm16[:, hs], mcb[h][:], ALU.subtract
                    )
                db = db16[:, None, :].to_broadcast([P, HC, F])
                rb = rb16[:, None, :].to_broadcast([P, HC, F])
                for p in range(2):
                    t0 = c * TPC + p * HC
                    ysl = y16[:, t0 : t0 + HC, :]
                    nc.vector.tensor_tensor(ysl, ysl, db, ALU.subtract)
                    nc.vector.tensor_tensor(ysl, ysl, rb, ALU.mult)
                    for j in range(HC):
                        t = t0 + j
                        nc.sync.dma_start(out=out_t[t], in_=y16[:, t, :])

    nc.finalize()
    return nc


@functools.cache
def _get_nc():
    return build_kernel()


def kernel(x, M, S, _trace=False, _trace_kwargs=None):
    del M  # overwritten by the first Welford step in the reference
    x = np.ascontiguousarray(x, dtype=np.float32)
    S = np.ascontiguousarray(S, dtype=np.float32).reshape(1, F)
    nc = _get_nc()
    in_maps = [
        {"x": x[i * ROWS : (i + 1) * ROWS], "S": S} for i in range(NCORES)
    ]
    res = run_bass_kernel_spmd(
        nc,
        in_maps,
        core_ids=list(range(NCORES)),
        trace=_trace,
        **(_trace_kwargs or {}),
    )
    out = np.concatenate(
        [res.results[i]["out"] for i in range(NCORES)], axis=0
    ).astype(np.float32)
    if _trace:
        return out, res
    return out
